# revision 14
# baseline (speedup 1.0000x reference)
"""Multi-head attention (B=4, N=1024, D=1024, 16 heads x 64) on 8 TRN2 cores.

Sharding: core c -> (batch b = c//2, head-group g = c%2). Each core computes
attention for 8 heads of one batch plus its slice of the output projection
(Wo row-parallel); host sums the two head-group partials per batch.

Schedule (v3):
- DMA order: (xt,wv)x8, wq x8, wk x8, binm x8, wo — V projection only needs
  the stream head; attention starts right after wk lands.
- PSUM rings: st pool (2 x 2 banks) serves the early V-projection double
  tiles then the attention score tiles; u pool (2 x 1 bank) holds only the
  softmax accumulators; t pool (2 x 1 bank) takes every transient group
  (QKV projections, prefetch, output projection).
- Each engine executes its stream in order, so all filler work (QK prefetch
  for the next pair, proj(0) groups) is hooked late in each attention block
  (jc3/jc6) where the exp pipeline otherwise gates the PE.
- proj(0) overlaps attention(1,3); proj(1) drains on alternating psum rings
  with stages alternating Scalar/Vector engines. Output is bf16.
"""

from contextlib import ExitStack, nullcontext

import ml_dtypes
import numpy as np

import concourse.bass as bass
import concourse.mybir as mybir
import concourse.tile as tile
from concourse import bacc
from concourse.bass_utils import run_bass_kernel_spmd

B, N, D = 4, 1024, 1024
HEADS, DH = 16, 64
SCALE = DH ** -0.5
NCORES = 8
HPC = HEADS // 2          # heads per core = 8
IPC = HPC * DH            # inner slice per core = 512
P = 128
IC = 512                  # i (query position) chunk = max psum free dim
NI = N // IC              # 2
NJ = N // P               # 8 key-position chunks
NKC = D // P              # 8 contraction chunks for projections
NMC = IPC // P            # 4 inner chunks per core
NDO = D // P              # 8 output-dim chunks

F32 = mybir.dt.float32
F32R = mybir.dt.float32r
BF16 = mybir.dt.bfloat16
EXP = mybir.ActivationFunctionType.Exp


def _r(ap):
    return ap if ap.dtype == F32R else ap.bitcast(F32R)


def _build(loop=1):
    nc = bacc.Bacc("TRN2", target_bir_lowering=False, debug=False)
    xT = nc.dram_tensor("xT", [D, N], BF16, kind="ExternalInput")
    wq = nc.dram_tensor("wq", [D, IPC], BF16, kind="ExternalInput")
    wk = nc.dram_tensor("wk", [D, IPC], BF16, kind="ExternalInput")
    wv = nc.dram_tensor("wv", [D, IPC], BF16, kind="ExternalInput")
    wo = nc.dram_tensor("wo", [IPC, D], BF16, kind="ExternalInput")
    binmT = nc.dram_tensor("binmT", [N, N], BF16, kind="ExternalInput")
    outT = nc.dram_tensor("outT", [D, N], BF16, kind="ExternalOutput")

    xT_r = xT.rearrange("(kc p) n -> kc p n", p=P)
    wq_r = wq.rearrange("(kc p) m -> kc p m", p=P)
    wk_r = wk.rearrange("(kc p) m -> kc p m", p=P)
    wv_r = wv.rearrange("(kc p) m -> kc p m", p=P)
    wo_r = wo.rearrange("(kc p) m -> kc p m", p=P)
    binmT_r = binmT.rearrange("(jc p) i -> p jc i", p=P)
    outT_r = outT.rearrange("(do p) n -> do p n", p=P)

    with tile.TileContext(nc) as tc, ExitStack() as ctx:
        xt_pool = ctx.enter_context(tc.tile_pool(name="xt", bufs=2))
        w_pool = ctx.enter_context(tc.tile_pool(name="w", bufs=5))
        qk_pool = ctx.enter_context(tc.tile_pool(name="qk", bufs=2))
        v_pool = ctx.enter_context(tc.tile_pool(name="v", bufs=2))
        m_pool = ctx.enter_context(tc.tile_pool(name="m", bufs=2))
        p_pool = ctx.enter_context(tc.tile_pool(name="p", bufs=6))
        ot_pool = ctx.enter_context(tc.tile_pool(name="ot", bufs=4))
        stage_pool = ctx.enter_context(tc.tile_pool(name="stage", bufs=4))
        small_pool = ctx.enter_context(tc.tile_pool(name="small", bufs=3))
        bc_pool = ctx.enter_context(tc.tile_pool(name="bc", bufs=2))
        # PSUM: st 2x(2 banks) + u 2x(1 bank) + t 2x(1 bank) = 8 banks
        psum_st = ctx.enter_context(tc.tile_pool(name="st", bufs=2, space="PSUM"))
        psum_u = ctx.enter_context(tc.tile_pool(name="u", bufs=2, space="PSUM"))
        psum_t = ctx.enter_context(tc.tile_pool(name="t", bufs=2, space="PSUM"))

        if loop > 1:
            loop_cm = tc.For_i(0, loop, 1)
        else:
            loop_cm = None
        with (loop_cm if loop_cm is not None else nullcontext()):
            # ---------------- DMA in ----------------
            xt_s = xt_pool.tile([P, NKC, N], BF16, tag="xt")
            wq_s = w_pool.tile([P, NKC, IPC], BF16, tag="w")
            wk_s = w_pool.tile([P, NKC, IPC], BF16, tag="w")
            wv_s = w_pool.tile([P, NKC, IPC], BF16, tag="w")
            for kc in range(NKC):
                nc.sync.dma_start(xt_s[:, kc, :], xT_r[kc])
                nc.sync.dma_start(wv_s[:, kc, :], wv_r[kc])
            for kc in range(NKC):
                nc.sync.dma_start(wq_s[:, kc, :], wq_r[kc])
            for kc in range(NKC):
                nc.sync.dma_start(wk_s[:, kc, :], wk_r[kc])
            binm_s = m_pool.tile([P, NJ, N], BF16, tag="binm")
            for jc in range(NJ):
                nc.sync.dma_start(binm_s[:, jc, :], binmT_r[:, jc, :])

            qt_s = qk_pool.tile([P, NMC, N], BF16, tag="qt")
            kt_s = qk_pool.tile([P, NMC, N], BF16, tag="kt")
            v_s = v_pool.tile([P, NJ, HPC, DH + 1], BF16, tag="v")
            ones_col = small_pool.tile([P, 1], BF16, tag="onescol")
            nc.vector.memset(ones_col[:], 1.0)
            # trigger the exp ACT-table load (~1.3us) during the DMA phase
            warm = small_pool.tile([1, 1], F32, tag="actwarm")
            nc.scalar.activation(warm, ones_col[:1, :1], EXP)
            nc.vector.tensor_copy(
                v_s[:, :, :, DH],
                ones_col[:, :, None].to_broadcast([P, NJ, HPC]),
            )

            # wo DMA issued after binm; first use is proj(0) much later.
            wo_s = w_pool.tile([P, NMC, D], BF16, tag="w")
            for kc in range(NMC):
                nc.sync.dma_start(wo_s[:, kc, :], wo_r[kc])

            # ---------------- compute emitters ----------------
            def emit_v_dbl(t):
                """V projection for key blocks 2t and 2t+1 in one st-ring
                double tile (the st ring is otherwise idle pre-attention)."""
                pv = psum_st.tile([P, 2, IPC], F32, tag="st", name=f"pv_{t}")
                for two in range(2):
                    for kc in range(NKC):
                        nc.tensor.matmul(
                            pv[:, two],
                            lhsT=xt_s[:, kc, (2 * t + two) * P:(2 * t + two + 1) * P],
                            rhs=wv_s[:, kc, :],
                            start=(kc == 0),
                            stop=(kc == NKC - 1),
                        )
                nc.scalar.copy(
                    v_s[:, 2 * t:2 * t + 2, :, :DH],
                    pv.rearrange("p two (h d) -> p two h d", h=HPC),
                )

            def emit_qk_grp(m, i, which, copy_eng="dve"):
                """One projection group: qt or kt for pair m, block i."""
                isl = slice(i * IC, (i + 1) * IC)
                w_s, dst = (wq_s, qt_s) if which == "q" else (wk_s, kt_s)
                pg = psum_t.tile([P, IC], F32, tag="t", name=f"p{which}_{m}_{i}")
                for kc in range(NKC):
                    nc.tensor.matmul(
                        pg,
                        lhsT=w_s[:, kc, m * P:(m + 1) * P],
                        rhs=xt_s[:, kc, isl],
                        start=(kc == 0),
                        stop=(kc == NKC - 1),
                    )
                if copy_eng == "act":
                    nc.scalar.copy(dst[:, m, isl], pg)
                else:
                    nc.vector.tensor_copy(dst[:, m, isl], pg)

            ots = []
            for i in range(NI):
                ots.append(
                    ot_pool.tile([P, NMC, IC], BF16, tag="ot", name=f"ot_{i}")
                )

            def emit_attention(i, pair, at_jc=None):
                isl = slice(i * IC, (i + 1) * IC)
                ot = ots[i]
                us = [
                    psum_u.tile([P, IC], F32, tag="u", name=f"u_{i}_{pair}_{h}")
                    for h in range(2)
                ]
                for jc in range(NJ):
                    if at_jc is not None and jc in at_jc:
                        at_jc[jc]()
                    jsl = slice(jc * P, (jc + 1) * P)
                    st = psum_st.tile([P, 2, IC], F32, tag="st")
                    for half in range(2):
                        hsl = slice(half * DH, (half + 1) * DH)
                        nc.tensor.matmul(
                            st[:, half],
                            lhsT=kt_s[hsl, pair, jsl],
                            rhs=qt_s[hsl, pair, isl],
                            start=True,
                            stop=True,
                        )
                    p_t = p_pool.tile([P, 2, IC], BF16, tag="p")
                    nc.scalar.activation(p_t, st, EXP)
                    nc.vector.tensor_mul(
                        out=p_t, in0=p_t,
                        in1=binm_s[:, jc, None, isl].to_broadcast([P, 2, IC]),
                    )
                    for half in range(2):
                        h = 2 * pair + half
                        nc.tensor.matmul(
                            us[half][: DH + 1],
                            lhsT=v_s[:, jc, h, :],
                            rhs=p_t[:, half],
                            start=(jc == 0),
                            stop=(jc == NJ - 1),
                        )
                for half in range(2):
                    u = us[half]
                    rsum = small_pool.tile([1, IC], F32, tag="rsum")
                    nc.vector.tensor_copy(rsum, u[DH:DH + 1, :])
                    rr = small_pool.tile([1, IC], F32, tag="rr")
                    nc.vector.reciprocal_approx_fast(out=rr, in_=rsum)
                    bcs = bc_pool.tile([DH, IC], F32, tag="bcs")
                    nc.gpsimd.partition_broadcast(bcs, rr)
                    nc.vector.tensor_mul(
                        out=ot[half * DH:(half + 1) * DH, pair, :],
                        in0=u[:DH, :],
                        in1=bcs,
                    )

            def emit_proj_grp(i, do, stage_eng="act", ring="t"):
                isl = slice(i * IC, (i + 1) * IC)
                pool = psum_t if ring == "t" else psum_u
                pr = pool.tile([P, IC], F32, tag=("t" if ring == "t" else "u"),
                               name=f"pr_{i}_{do}")
                for kc in range(NMC):
                    nc.tensor.matmul(
                        pr,
                        lhsT=wo_s[:, kc, do * P:(do + 1) * P],
                        rhs=ots[i][:, kc, :],
                        start=(kc == 0),
                        stop=(kc == NMC - 1),
                    )
                stg = stage_pool.tile([P, IC], BF16, tag="stg")
                if stage_eng == "act":
                    nc.scalar.copy(stg, pr)
                else:
                    nc.vector.tensor_copy(stg, pr)
                nc.sync.dma_start(outT_r[do][:, isl], stg)

            # ---------------- schedule ----------------
            # Early phase: V doubles (st ring) interleaved with pair-0 QKV
            # groups (t ring); the emission order tracks the DMA stream.
            emit_v_dbl(0)
            emit_v_dbl(1)
            emit_v_dbl(2)
            emit_qk_grp(0, 0, "q", copy_eng="act")
            emit_v_dbl(3)
            emit_qk_grp(0, 1, "q", copy_eng="act")
            emit_qk_grp(0, 0, "k", copy_eng="act")
            emit_qk_grp(0, 1, "k", copy_eng="act")

            for pair in range(NMC):
                for i in range(NI):
                    hooks = {}
                    if pair + 1 < NMC:
                        hooks[4] = lambda m=pair + 1, ii=i: emit_qk_grp(m, ii, "q")
                        hooks[7] = lambda m=pair + 1, ii=i: emit_qk_grp(m, ii, "k")
                    elif i == 1:
                        # last block: proj(0) groups as late fillers
                        for jc, do in zip((2, 3, 4, 5, 6, 7), range(6)):
                            eng = "dve" if do < 4 else "act"
                            hooks[jc] = lambda d=do, e=eng: emit_proj_grp(0, d, e, "t")
                    emit_attention(i, pair, at_jc=hooks)
            for do in range(6, NDO):
                emit_proj_grp(0, do, "act", "t")
            for do in range(NDO):
                emit_proj_grp(1, do, "act" if do % 2 == 0 else "dve",
                              "t" if do % 2 == 0 else "u")

    nc.compile()
    return nc


_nc_cache = {}


def _get_nc(loop=1):
    if loop not in _nc_cache:
        _nc_cache[loop] = _build(loop)
    return _nc_cache[loop]


_last_results = [None]
_last_in_maps = [None]


def kernel(x, mask, Wq, Wk, Wv, Wo, bo):
    x = np.asarray(x, dtype=np.float32)
    mask = np.asarray(mask)
    Wq = np.asarray(Wq, dtype=np.float32)
    Wk = np.asarray(Wk, dtype=np.float32)
    Wv = np.asarray(Wv, dtype=np.float32)
    Wo = np.asarray(Wo, dtype=np.float32)
    bo = np.asarray(bo, dtype=np.float32)

    nc = _get_nc()
    in_maps = []
    for c in range(NCORES):
        b, g = divmod(c, 2)
        gsl = slice(g * IPC, (g + 1) * IPC)
        keep = (mask[b, 0] == 0).T
        in_maps.append(
            {
                "xT": np.ascontiguousarray(x[b].T.astype(ml_dtypes.bfloat16)),
                "wq": np.ascontiguousarray((Wq[:, gsl] * np.float32(SCALE)).astype(ml_dtypes.bfloat16)),
                "wk": np.ascontiguousarray(Wk[:, gsl].astype(ml_dtypes.bfloat16)),
                "wv": np.ascontiguousarray(Wv[:, gsl].astype(ml_dtypes.bfloat16)),
                "wo": np.ascontiguousarray(Wo[gsl, :].astype(ml_dtypes.bfloat16)),
                "binmT": np.ascontiguousarray(keep.astype(ml_dtypes.bfloat16)),
            }
        )
    _last_in_maps[0] = in_maps
    res = run_bass_kernel_spmd(nc, in_maps, core_ids=list(range(NCORES)))
    _last_results[0] = res
    outs = [np.asarray(r["outT"], dtype=np.float32) for r in res.results]
    out = np.empty((B, N, D), np.float32)
    for b in range(B):
        out[b] = (outs[2 * b] + outs[2 * b + 1]).T + bo
    return out


# revision 15
# speedup vs baseline: 1.0376x; 1.0376x over previous
"""Multi-head attention (B=4, N=1024, D=1024, 16 heads x 64) on 8 TRN2 cores.

Sharding: core c -> (batch b = c//2, head-group g = c%2). Each core computes
attention for 8 heads of one batch plus its slice of the output projection
(Wo row-parallel); host sums the two head-group partials per batch.

Schedule (v3):
- DMA order: (xt,wv)x8, wq x8, wk x8, binm x8, wo — V projection only needs
  the stream head; attention starts right after wk lands.
- PSUM rings: st pool (2 x 2 banks) serves the early V-projection double
  tiles then the attention score tiles; u pool (2 x 1 bank) holds only the
  softmax accumulators; t pool (2 x 1 bank) takes every transient group
  (QKV projections, prefetch, output projection).
- Each engine executes its stream in order, so all filler work (QK prefetch
  for the next pair, proj(0) groups) is hooked late in each attention block
  (jc3/jc6) where the exp pipeline otherwise gates the PE.
- proj(0) overlaps attention(1,3); proj(1) drains on alternating psum rings
  with stages alternating Scalar/Vector engines. Output is bf16.
"""

from contextlib import ExitStack, nullcontext

import ml_dtypes
import numpy as np

import concourse.bass as bass
import concourse.mybir as mybir
import concourse.tile as tile
from concourse import bacc
from concourse.bass_utils import run_bass_kernel_spmd

B, N, D = 4, 1024, 1024
HEADS, DH = 16, 64
SCALE = DH ** -0.5
NCORES = 8
HPC = HEADS // 2          # heads per core = 8
IPC = HPC * DH            # inner slice per core = 512
P = 128
IC = 512                  # i (query position) chunk = max psum free dim
NI = N // IC              # 2
NJ = N // P               # 8 key-position chunks
NKC = D // P              # 8 contraction chunks for projections
NMC = IPC // P            # 4 inner chunks per core
NDO = D // P              # 8 output-dim chunks

F32 = mybir.dt.float32
F32R = mybir.dt.float32r
BF16 = mybir.dt.bfloat16
EXP = mybir.ActivationFunctionType.Exp


def _r(ap):
    return ap if ap.dtype == F32R else ap.bitcast(F32R)


def _build(loop=1):
    nc = bacc.Bacc("TRN2", target_bir_lowering=False, debug=False)
    xT = nc.dram_tensor("xT", [D, N], BF16, kind="ExternalInput")
    wq = nc.dram_tensor("wq", [D, IPC], BF16, kind="ExternalInput")
    wk = nc.dram_tensor("wk", [D, IPC], BF16, kind="ExternalInput")
    wv = nc.dram_tensor("wv", [D, IPC], BF16, kind="ExternalInput")
    wo = nc.dram_tensor("wo", [IPC, D], BF16, kind="ExternalInput")
    binmT = nc.dram_tensor("binmT", [N, N], BF16, kind="ExternalInput")
    outT = nc.dram_tensor("outT", [D, N], BF16, kind="ExternalOutput")

    xT_r = xT.rearrange("(kc p) n -> kc p n", p=P)
    wq_r = wq.rearrange("(kc p) m -> kc p m", p=P)
    wk_r = wk.rearrange("(kc p) m -> kc p m", p=P)
    wv_r = wv.rearrange("(kc p) m -> kc p m", p=P)
    wo_r = wo.rearrange("(kc p) m -> kc p m", p=P)
    binmT_r = binmT.rearrange("(jc p) i -> p jc i", p=P)
    outT_r = outT.rearrange("(do p) n -> do p n", p=P)

    with tile.TileContext(nc) as tc, ExitStack() as ctx:
        xt_pool = ctx.enter_context(tc.tile_pool(name="xt", bufs=2))
        w_pool = ctx.enter_context(tc.tile_pool(name="w", bufs=5))
        qk_pool = ctx.enter_context(tc.tile_pool(name="qk", bufs=2))
        v_pool = ctx.enter_context(tc.tile_pool(name="v", bufs=2))
        m_pool = ctx.enter_context(tc.tile_pool(name="m", bufs=2))
        p_pool = ctx.enter_context(tc.tile_pool(name="p", bufs=6))
        ot_pool = ctx.enter_context(tc.tile_pool(name="ot", bufs=4))
        stage_pool = ctx.enter_context(tc.tile_pool(name="stage", bufs=4))
        small_pool = ctx.enter_context(tc.tile_pool(name="small", bufs=3))
        bc_pool = ctx.enter_context(tc.tile_pool(name="bc", bufs=2))
        # PSUM: st 2x(2 banks) + u 2x(1 bank) + t 2x(1 bank) = 8 banks
        psum_st = ctx.enter_context(tc.tile_pool(name="st", bufs=2, space="PSUM"))
        psum_u = ctx.enter_context(tc.tile_pool(name="u", bufs=2, space="PSUM"))
        psum_t = ctx.enter_context(tc.tile_pool(name="t", bufs=2, space="PSUM"))

        if loop > 1:
            loop_cm = tc.For_i(0, loop, 1)
        else:
            loop_cm = None
        with (loop_cm if loop_cm is not None else nullcontext()):
            # ---------------- DMA in ----------------
            xt_s = xt_pool.tile([P, NKC, N], BF16, tag="xt")
            wq_s = w_pool.tile([P, NKC, IPC], BF16, tag="w")
            wk_s = w_pool.tile([P, NKC, IPC], BF16, tag="w")
            wv_s = w_pool.tile([P, NKC, IPC], BF16, tag="w")
            for kc in range(NKC):
                nc.sync.dma_start(xt_s[:, kc, :], xT_r[kc])
                nc.sync.dma_start(wv_s[:, kc, :], wv_r[kc])
            for kc in range(NKC):
                nc.sync.dma_start(wq_s[:, kc, :], wq_r[kc])
            for kc in range(NKC):
                nc.sync.dma_start(wk_s[:, kc, :], wk_r[kc])
            binm_s = m_pool.tile([P, NJ, N], BF16, tag="binm")
            for jc in range(NJ):
                nc.sync.dma_start(binm_s[:, jc, :], binmT_r[:, jc, :])

            qt_s = qk_pool.tile([P, NMC, N], BF16, tag="qt")
            kt_s = qk_pool.tile([P, NMC, N], BF16, tag="kt")
            v_s = v_pool.tile([P, NJ, HPC, DH + 1], BF16, tag="v")
            ones_col = small_pool.tile([P, 1], BF16, tag="onescol")
            nc.vector.memset(ones_col[:], 1.0)
            # trigger the exp ACT-table load (~1.3us) during the DMA phase
            warm = small_pool.tile([1, 1], F32, tag="actwarm")
            nc.scalar.activation(warm, ones_col[:1, :1], EXP)
            nc.vector.tensor_copy(
                v_s[:, :, :, DH],
                ones_col[:, :, None].to_broadcast([P, NJ, HPC]),
            )

            # wo DMA issued after binm; first use is proj(0) much later.
            wo_s = w_pool.tile([P, NMC, D], BF16, tag="w")
            for kc in range(NMC):
                nc.sync.dma_start(wo_s[:, kc, :], wo_r[kc])

            # ---------------- compute emitters ----------------
            def emit_v_dbl(t):
                """V projection for key blocks 2t and 2t+1 in one st-ring
                double tile (the st ring is otherwise idle pre-attention)."""
                pv = psum_st.tile([P, 2, IPC], F32, tag="st", name=f"pv_{t}")
                for two in range(2):
                    for kc in range(NKC):
                        nc.tensor.matmul(
                            pv[:, two],
                            lhsT=xt_s[:, kc, (2 * t + two) * P:(2 * t + two + 1) * P],
                            rhs=wv_s[:, kc, :],
                            start=(kc == 0),
                            stop=(kc == NKC - 1),
                        )
                nc.scalar.copy(
                    v_s[:, 2 * t:2 * t + 2, :, :DH],
                    pv.rearrange("p two (h d) -> p two h d", h=HPC),
                )

            def emit_qk_grp(m, i, which, copy_eng="dve"):
                """One projection group: qt or kt for pair m, block i."""
                isl = slice(i * IC, (i + 1) * IC)
                w_s, dst = (wq_s, qt_s) if which == "q" else (wk_s, kt_s)
                pg = psum_t.tile([P, IC], F32, tag="t", name=f"p{which}_{m}_{i}")
                for kc in range(NKC):
                    nc.tensor.matmul(
                        pg,
                        lhsT=w_s[:, kc, m * P:(m + 1) * P],
                        rhs=xt_s[:, kc, isl],
                        start=(kc == 0),
                        stop=(kc == NKC - 1),
                    )
                if copy_eng == "act":
                    nc.scalar.copy(dst[:, m, isl], pg)
                else:
                    nc.vector.tensor_copy(dst[:, m, isl], pg)

            ots = []
            for i in range(NI):
                ots.append(
                    ot_pool.tile([P, NMC, IC], BF16, tag="ot", name=f"ot_{i}")
                )

            def emit_attention(i, pair, at_jc=None):
                isl = slice(i * IC, (i + 1) * IC)
                ot = ots[i]
                us = [
                    psum_u.tile([P, IC], F32, tag="u", name=f"u_{i}_{pair}_{h}")
                    for h in range(2)
                ]
                for jc in range(NJ):
                    if at_jc is not None and jc in at_jc:
                        at_jc[jc]()
                    jsl = slice(jc * P, (jc + 1) * P)
                    st = psum_st.tile([P, 2, IC], F32, tag="st")
                    for half in range(2):
                        hsl = slice(half * DH, (half + 1) * DH)
                        nc.tensor.matmul(
                            st[:, half],
                            lhsT=kt_s[hsl, pair, jsl],
                            rhs=qt_s[hsl, pair, isl],
                            start=True,
                            stop=True,
                        )
                    p_t = p_pool.tile([P, 2, IC], BF16, tag="p")
                    nc.scalar.activation(p_t, st, EXP)
                    nc.vector.tensor_mul(
                        out=p_t, in0=p_t,
                        in1=binm_s[:, jc, None, isl].to_broadcast([P, 2, IC]),
                    )
                    for half in range(2):
                        h = 2 * pair + half
                        nc.tensor.matmul(
                            us[half][: DH + 1],
                            lhsT=v_s[:, jc, h, :],
                            rhs=p_t[:, half],
                            start=(jc == 0),
                            stop=(jc == NJ - 1),
                        )
                for half in range(2):
                    u = us[half]
                    rsum = small_pool.tile([1, IC], F32, tag="rsum")
                    nc.vector.tensor_copy(rsum, u[DH:DH + 1, :])
                    rr = small_pool.tile([1, IC], F32, tag="rr")
                    nc.vector.reciprocal_approx_fast(out=rr, in_=rsum)
                    bcs = bc_pool.tile([DH, IC], F32, tag="bcs")
                    nc.gpsimd.partition_broadcast(bcs, rr)
                    nc.vector.tensor_mul(
                        out=ot[half * DH:(half + 1) * DH, pair, :],
                        in0=u[:DH, :],
                        in1=bcs,
                    )

            def emit_proj_grp(i, do, stage_eng="act", ring="t"):
                isl = slice(i * IC, (i + 1) * IC)
                pool = psum_t if ring == "t" else psum_u
                pr = pool.tile([P, IC], F32, tag=("t" if ring == "t" else "u"),
                               name=f"pr_{i}_{do}")
                for kc in range(NMC):
                    nc.tensor.matmul(
                        pr,
                        lhsT=wo_s[:, kc, do * P:(do + 1) * P],
                        rhs=ots[i][:, kc, :],
                        start=(kc == 0),
                        stop=(kc == NMC - 1),
                    )
                stg = stage_pool.tile([P, IC], BF16, tag="stg")
                if stage_eng == "act":
                    nc.scalar.copy(stg, pr)
                else:
                    nc.vector.tensor_copy(stg, pr)
                nc.sync.dma_start(outT_r[do][:, isl], stg)

            # ---------------- schedule ----------------
            # Early phase: V doubles (st ring) interleaved with pair-0 QKV
            # groups (t ring); the emission order tracks the DMA stream.
            emit_v_dbl(0)
            emit_v_dbl(1)
            emit_v_dbl(2)
            emit_qk_grp(0, 0, "q", copy_eng="act")
            emit_v_dbl(3)
            emit_qk_grp(0, 1, "q", copy_eng="act")
            emit_qk_grp(0, 0, "k", copy_eng="act")
            emit_qk_grp(0, 1, "k", copy_eng="act")

            for pair in range(NMC):
                for i in range(NI):
                    hooks = {}
                    if pair + 1 < NMC:
                        hooks[4] = lambda m=pair + 1, ii=i: emit_qk_grp(m, ii, "q")
                        hooks[7] = lambda m=pair + 1, ii=i: emit_qk_grp(m, ii, "k")
                    elif i == 1:
                        # last block: proj(0) groups as late fillers
                        for jc, do in zip((2, 3, 4, 5, 6, 7), range(6)):
                            hooks[jc] = lambda d=do: emit_proj_grp(0, d, "dve", "t")
                    emit_attention(i, pair, at_jc=hooks)
            for do in range(6, NDO):
                emit_proj_grp(0, do, "act", "t")
            for do in range(NDO):
                emit_proj_grp(1, do, "act" if do % 2 == 0 else "dve",
                              "t" if do % 2 == 0 else "u")

    nc.compile()
    return nc


_nc_cache = {}


def _get_nc(loop=1):
    if loop not in _nc_cache:
        _nc_cache[loop] = _build(loop)
    return _nc_cache[loop]


_last_results = [None]
_last_in_maps = [None]


def kernel(x, mask, Wq, Wk, Wv, Wo, bo):
    x = np.asarray(x, dtype=np.float32)
    mask = np.asarray(mask)
    Wq = np.asarray(Wq, dtype=np.float32)
    Wk = np.asarray(Wk, dtype=np.float32)
    Wv = np.asarray(Wv, dtype=np.float32)
    Wo = np.asarray(Wo, dtype=np.float32)
    bo = np.asarray(bo, dtype=np.float32)

    nc = _get_nc()
    in_maps = []
    for c in range(NCORES):
        b, g = divmod(c, 2)
        gsl = slice(g * IPC, (g + 1) * IPC)
        keep = (mask[b, 0] == 0).T
        in_maps.append(
            {
                "xT": np.ascontiguousarray(x[b].T.astype(ml_dtypes.bfloat16)),
                "wq": np.ascontiguousarray((Wq[:, gsl] * np.float32(SCALE)).astype(ml_dtypes.bfloat16)),
                "wk": np.ascontiguousarray(Wk[:, gsl].astype(ml_dtypes.bfloat16)),
                "wv": np.ascontiguousarray(Wv[:, gsl].astype(ml_dtypes.bfloat16)),
                "wo": np.ascontiguousarray(Wo[gsl, :].astype(ml_dtypes.bfloat16)),
                "binmT": np.ascontiguousarray(keep.astype(ml_dtypes.bfloat16)),
            }
        )
    _last_in_maps[0] = in_maps
    res = run_bass_kernel_spmd(nc, in_maps, core_ids=list(range(NCORES)))
    _last_results[0] = res
    outs = [np.asarray(r["outT"], dtype=np.float32) for r in res.results]
    out = np.empty((B, N, D), np.float32)
    for b in range(B):
        out[b] = (outs[2 * b] + outs[2 * b + 1]).T + bo
    return out


# revision 16
# speedup vs baseline: 1.0794x; 1.0403x over previous
"""Multi-head attention (B=4, N=1024, D=1024, 16 heads x 64) on 8 TRN2 cores.

Sharding: core c -> (batch b = c//2, head-group g = c%2). Each core computes
attention for 8 heads of one batch plus its slice of the output projection
(Wo row-parallel); host sums the two head-group partials per batch.

Schedule (v3):
- DMA order: (xt,wv)x8, wq x8, wk x8, binm x8, wo — V projection only needs
  the stream head; attention starts right after wk lands.
- PSUM rings: st pool (2 x 2 banks) serves the early V-projection double
  tiles then the attention score tiles; u pool (2 x 1 bank) holds only the
  softmax accumulators; t pool (2 x 1 bank) takes every transient group
  (QKV projections, prefetch, output projection).
- Each engine executes its stream in order, so all filler work (QK prefetch
  for the next pair, proj(0) groups) is hooked late in each attention block
  (jc3/jc6) where the exp pipeline otherwise gates the PE.
- proj(0) overlaps attention(1,3); proj(1) drains on alternating psum rings
  with stages alternating Scalar/Vector engines. Output is bf16.
"""

from contextlib import ExitStack, nullcontext

import ml_dtypes
import numpy as np

import concourse.bass as bass
import concourse.mybir as mybir
import concourse.tile as tile
from concourse import bacc
from concourse.bass_utils import run_bass_kernel_spmd

B, N, D = 4, 1024, 1024
HEADS, DH = 16, 64
SCALE = DH ** -0.5
NCORES = 8
HPC = HEADS // 2          # heads per core = 8
IPC = HPC * DH            # inner slice per core = 512
P = 128
IC = 512                  # i (query position) chunk = max psum free dim
NI = N // IC              # 2
NJ = N // P               # 8 key-position chunks
NKC = D // P              # 8 contraction chunks for projections
NMC = IPC // P            # 4 inner chunks per core
NDO = D // P              # 8 output-dim chunks

F32 = mybir.dt.float32
F32R = mybir.dt.float32r
BF16 = mybir.dt.bfloat16
EXP = mybir.ActivationFunctionType.Exp


def _r(ap):
    return ap if ap.dtype == F32R else ap.bitcast(F32R)


def _build(loop=1):
    nc = bacc.Bacc("TRN2", target_bir_lowering=False, debug=False)
    xT = nc.dram_tensor("xT", [D, N], BF16, kind="ExternalInput")
    wq = nc.dram_tensor("wq", [D, IPC], BF16, kind="ExternalInput")
    wk = nc.dram_tensor("wk", [D, IPC], BF16, kind="ExternalInput")
    wv = nc.dram_tensor("wv", [D, IPC], BF16, kind="ExternalInput")
    wo = nc.dram_tensor("wo", [IPC, D], BF16, kind="ExternalInput")
    binmT = nc.dram_tensor("binmT", [N, N], BF16, kind="ExternalInput")
    outT = nc.dram_tensor("outT", [D, N], BF16, kind="ExternalOutput")

    xT_r = xT.rearrange("(kc p) n -> kc p n", p=P)
    wq_r = wq.rearrange("(kc p) m -> kc p m", p=P)
    wk_r = wk.rearrange("(kc p) m -> kc p m", p=P)
    wv_r = wv.rearrange("(kc p) m -> kc p m", p=P)
    wo_r = wo.rearrange("(kc p) m -> kc p m", p=P)
    binmT_r = binmT.rearrange("(jc p) i -> p jc i", p=P)
    outT_r = outT.rearrange("(do p) n -> do p n", p=P)

    with tile.TileContext(nc) as tc, ExitStack() as ctx:
        xt_pool = ctx.enter_context(tc.tile_pool(name="xt", bufs=2))
        w_pool = ctx.enter_context(tc.tile_pool(name="w", bufs=5))
        qk_pool = ctx.enter_context(tc.tile_pool(name="qk", bufs=2))
        v_pool = ctx.enter_context(tc.tile_pool(name="v", bufs=2))
        m_pool = ctx.enter_context(tc.tile_pool(name="m", bufs=2))
        p_pool = ctx.enter_context(tc.tile_pool(name="p", bufs=6))
        ot_pool = ctx.enter_context(tc.tile_pool(name="ot", bufs=4))
        stage_pool = ctx.enter_context(tc.tile_pool(name="stage", bufs=4))
        small_pool = ctx.enter_context(tc.tile_pool(name="small", bufs=3))
        bc_pool = ctx.enter_context(tc.tile_pool(name="bc", bufs=2))
        # PSUM: st 2x(2 banks) + u 2x(1 bank) + t 2x(1 bank) = 8 banks
        psum_st = ctx.enter_context(tc.tile_pool(name="st", bufs=2, space="PSUM"))
        psum_u = ctx.enter_context(tc.tile_pool(name="u", bufs=2, space="PSUM"))
        psum_t = ctx.enter_context(tc.tile_pool(name="t", bufs=2, space="PSUM"))

        if loop > 1:
            loop_cm = tc.For_i(0, loop, 1)
        else:
            loop_cm = None
        with (loop_cm if loop_cm is not None else nullcontext()):
            # ---------------- DMA in ----------------
            xt_s = xt_pool.tile([P, NKC, N], BF16, tag="xt")
            wq_s = w_pool.tile([P, NKC, IPC], BF16, tag="w")
            wk_s = w_pool.tile([P, NKC, IPC], BF16, tag="w")
            wv_s = w_pool.tile([P, NKC, IPC], BF16, tag="w")
            for kc in range(NKC):
                nc.sync.dma_start(xt_s[:, kc, :], xT_r[kc])
                nc.sync.dma_start(wv_s[:, kc, :], wv_r[kc])
            for kc in range(NKC):
                nc.sync.dma_start(wq_s[:, kc, :], wq_r[kc])
            for kc in range(NKC):
                nc.sync.dma_start(wk_s[:, kc, :], wk_r[kc])
            binm_s = m_pool.tile([P, NJ, N], BF16, tag="binm")
            for jc in range(NJ):
                nc.sync.dma_start(binm_s[:, jc, :], binmT_r[:, jc, :])

            qt_s = qk_pool.tile([P, NMC, N], BF16, tag="qt")
            kt_s = qk_pool.tile([P, NMC, N], BF16, tag="kt")
            v_s = v_pool.tile([P, NJ, HPC, DH + 1], BF16, tag="v")
            ones_col = small_pool.tile([P, 1], BF16, tag="onescol")
            nc.vector.memset(ones_col[:], 1.0)
            # trigger the exp ACT-table load (~1.3us) during the DMA phase
            warm = small_pool.tile([1, 1], F32, tag="actwarm")
            nc.scalar.activation(warm, ones_col[:1, :1], EXP)
            nc.vector.tensor_copy(
                v_s[:, :, :, DH],
                ones_col[:, :, None].to_broadcast([P, NJ, HPC]),
            )

            # wo DMA issued after binm; first use is proj(0) much later.
            wo_s = w_pool.tile([P, NMC, D], BF16, tag="w")
            for kc in range(NMC):
                nc.sync.dma_start(wo_s[:, kc, :], wo_r[kc])

            # ---------------- compute emitters ----------------
            def emit_v_dbl(t):
                """V projection for key blocks 2t and 2t+1 in one st-ring
                double tile (the st ring is otherwise idle pre-attention)."""
                pv = psum_st.tile([P, 2, IPC], F32, tag="st", name=f"pv_{t}")
                for two in range(2):
                    for kc in range(NKC):
                        nc.tensor.matmul(
                            pv[:, two],
                            lhsT=xt_s[:, kc, (2 * t + two) * P:(2 * t + two + 1) * P],
                            rhs=wv_s[:, kc, :],
                            start=(kc == 0),
                            stop=(kc == NKC - 1),
                        )
                nc.scalar.copy(
                    v_s[:, 2 * t:2 * t + 2, :, :DH],
                    pv.rearrange("p two (h d) -> p two h d", h=HPC),
                )

            def emit_qk_grp(m, i, which, copy_eng="dve"):
                """One projection group: qt or kt for pair m, block i."""
                isl = slice(i * IC, (i + 1) * IC)
                w_s, dst = (wq_s, qt_s) if which == "q" else (wk_s, kt_s)
                pg = psum_t.tile([P, IC], F32, tag="t", name=f"p{which}_{m}_{i}")
                for kc in range(NKC):
                    nc.tensor.matmul(
                        pg,
                        lhsT=w_s[:, kc, m * P:(m + 1) * P],
                        rhs=xt_s[:, kc, isl],
                        start=(kc == 0),
                        stop=(kc == NKC - 1),
                    )
                if copy_eng == "act":
                    nc.scalar.copy(dst[:, m, isl], pg)
                else:
                    nc.vector.tensor_copy(dst[:, m, isl], pg)

            ots = []
            for i in range(NI):
                ots.append(
                    ot_pool.tile([P, NMC, IC], BF16, tag="ot", name=f"ot_{i}")
                )

            def emit_attention(i, pair, at_jc=None):
                isl = slice(i * IC, (i + 1) * IC)
                ot = ots[i]
                us = [
                    psum_u.tile([P, IC], F32, tag="u", name=f"u_{i}_{pair}_{h}")
                    for h in range(2)
                ]
                for jc in range(NJ):
                    if at_jc is not None and jc in at_jc:
                        at_jc[jc]()
                    jsl = slice(jc * P, (jc + 1) * P)
                    st = psum_st.tile([P, 2, IC], F32, tag="st")
                    for half in range(2):
                        hsl = slice(half * DH, (half + 1) * DH)
                        nc.tensor.matmul(
                            st[:, half],
                            lhsT=kt_s[hsl, pair, jsl],
                            rhs=qt_s[hsl, pair, isl],
                            start=True,
                            stop=True,
                        )
                    p_t = p_pool.tile([P, 2, IC], BF16, tag="p")
                    nc.scalar.activation(p_t, st, EXP)
                    nc.vector.tensor_mul(
                        out=p_t, in0=p_t,
                        in1=binm_s[:, jc, None, isl].to_broadcast([P, 2, IC]),
                    )
                    for half in range(2):
                        h = 2 * pair + half
                        nc.tensor.matmul(
                            us[half][: DH + 1],
                            lhsT=v_s[:, jc, h, :],
                            rhs=p_t[:, half],
                            start=(jc == 0),
                            stop=(jc == NJ - 1),
                        )
                for half in range(2):
                    u = us[half]
                    rsum = small_pool.tile([1, IC], F32, tag="rsum")
                    nc.vector.tensor_copy(rsum, u[DH:DH + 1, :])
                    rr = small_pool.tile([1, IC], F32, tag="rr")
                    nc.vector.reciprocal_approx_fast(out=rr, in_=rsum)
                    bcs = bc_pool.tile([DH, IC], F32, tag="bcs")
                    nc.gpsimd.partition_broadcast(bcs, rr)
                    nc.vector.tensor_mul(
                        out=ot[half * DH:(half + 1) * DH, pair, :],
                        in0=u[:DH, :],
                        in1=bcs,
                    )

            def emit_proj_grp(i, do, stage_eng="act", ring="t"):
                isl = slice(i * IC, (i + 1) * IC)
                pool = psum_t if ring == "t" else psum_u
                pr = pool.tile([P, IC], F32, tag=("t" if ring == "t" else "u"),
                               name=f"pr_{i}_{do}")
                for kc in range(NMC):
                    nc.tensor.matmul(
                        pr,
                        lhsT=wo_s[:, kc, do * P:(do + 1) * P],
                        rhs=ots[i][:, kc, :],
                        start=(kc == 0),
                        stop=(kc == NMC - 1),
                    )
                stg = stage_pool.tile([P, IC], BF16, tag="stg")
                if stage_eng == "act":
                    nc.scalar.copy(stg, pr)
                else:
                    nc.vector.tensor_copy(stg, pr)
                # out-DMAs go on the Pool queue so SP's in-order stream stays
                # pure input supply: iteration k+1's loads can transfer while
                # iteration k is still draining.
                nc.gpsimd.dma_start(outT_r[do][:, isl], stg)

            # ---------------- schedule ----------------
            # Early phase: V doubles (st ring) interleaved with pair-0 QKV
            # groups (t ring); the emission order tracks the DMA stream.
            emit_v_dbl(0)
            emit_v_dbl(1)
            emit_v_dbl(2)
            emit_qk_grp(0, 0, "q", copy_eng="act")
            emit_v_dbl(3)
            emit_qk_grp(0, 1, "q", copy_eng="act")
            emit_qk_grp(0, 0, "k", copy_eng="act")
            emit_qk_grp(0, 1, "k", copy_eng="act")

            for pair in range(NMC):
                for i in range(NI):
                    hooks = {}
                    if pair + 1 < NMC:
                        hooks[4] = lambda m=pair + 1, ii=i: emit_qk_grp(m, ii, "q")
                        hooks[7] = lambda m=pair + 1, ii=i: emit_qk_grp(m, ii, "k")
                    elif i == 1:
                        # last block: proj(0) groups as late fillers
                        for jc, do in zip((2, 3, 4, 5, 6, 7), range(6)):
                            hooks[jc] = lambda d=do: emit_proj_grp(0, d, "dve", "t")
                    emit_attention(i, pair, at_jc=hooks)
            for do in range(6, NDO):
                emit_proj_grp(0, do, "act", "t")
            for do in range(NDO):
                emit_proj_grp(1, do, "act" if do % 2 == 0 else "dve",
                              "t" if do % 2 == 0 else "u")

    nc.compile()
    return nc


_nc_cache = {}


def _get_nc(loop=1):
    if loop not in _nc_cache:
        _nc_cache[loop] = _build(loop)
    return _nc_cache[loop]


_last_results = [None]
_last_in_maps = [None]


def kernel(x, mask, Wq, Wk, Wv, Wo, bo):
    x = np.asarray(x, dtype=np.float32)
    mask = np.asarray(mask)
    Wq = np.asarray(Wq, dtype=np.float32)
    Wk = np.asarray(Wk, dtype=np.float32)
    Wv = np.asarray(Wv, dtype=np.float32)
    Wo = np.asarray(Wo, dtype=np.float32)
    bo = np.asarray(bo, dtype=np.float32)

    nc = _get_nc()
    in_maps = []
    for c in range(NCORES):
        b, g = divmod(c, 2)
        gsl = slice(g * IPC, (g + 1) * IPC)
        keep = (mask[b, 0] == 0).T
        in_maps.append(
            {
                "xT": np.ascontiguousarray(x[b].T.astype(ml_dtypes.bfloat16)),
                "wq": np.ascontiguousarray((Wq[:, gsl] * np.float32(SCALE)).astype(ml_dtypes.bfloat16)),
                "wk": np.ascontiguousarray(Wk[:, gsl].astype(ml_dtypes.bfloat16)),
                "wv": np.ascontiguousarray(Wv[:, gsl].astype(ml_dtypes.bfloat16)),
                "wo": np.ascontiguousarray(Wo[gsl, :].astype(ml_dtypes.bfloat16)),
                "binmT": np.ascontiguousarray(keep.astype(ml_dtypes.bfloat16)),
            }
        )
    _last_in_maps[0] = in_maps
    res = run_bass_kernel_spmd(nc, in_maps, core_ids=list(range(NCORES)))
    _last_results[0] = res
    outs = [np.asarray(r["outT"], dtype=np.float32) for r in res.results]
    out = np.empty((B, N, D), np.float32)
    for b in range(B):
        out[b] = (outs[2 * b] + outs[2 * b + 1]).T + bo
    return out


# revision 17
# speedup vs baseline: 1.1033x; 1.0221x over previous
"""Multi-head attention (B=4, N=1024, D=1024, 16 heads x 64) on 8 TRN2 cores.

Sharding: core c -> (batch b = c//2, head-group g = c%2). Each core computes
attention for 8 heads of one batch plus its slice of the output projection
(Wo row-parallel); host sums the two head-group partials per batch.

Schedule (v3):
- DMA order: (xt,wv)x8, wq x8, wk x8, binm x8, wo — V projection only needs
  the stream head; attention starts right after wk lands.
- PSUM rings: st pool (2 x 2 banks) serves the early V-projection double
  tiles then the attention score tiles; u pool (2 x 1 bank) holds only the
  softmax accumulators; t pool (2 x 1 bank) takes every transient group
  (QKV projections, prefetch, output projection).
- Each engine executes its stream in order, so all filler work (QK prefetch
  for the next pair, proj(0) groups) is hooked late in each attention block
  (jc3/jc6) where the exp pipeline otherwise gates the PE.
- proj(0) overlaps attention(1,3); proj(1) drains on alternating psum rings
  with stages alternating Scalar/Vector engines. Output is bf16.
"""

from contextlib import ExitStack, nullcontext

import ml_dtypes
import numpy as np

import concourse.bass as bass
import concourse.mybir as mybir
import concourse.tile as tile
from concourse import bacc
from concourse.bass_utils import run_bass_kernel_spmd

B, N, D = 4, 1024, 1024
HEADS, DH = 16, 64
SCALE = DH ** -0.5
NCORES = 8
HPC = HEADS // 2          # heads per core = 8
IPC = HPC * DH            # inner slice per core = 512
P = 128
IC = 512                  # i (query position) chunk = max psum free dim
NI = N // IC              # 2
NJ = N // P               # 8 key-position chunks
NKC = D // P              # 8 contraction chunks for projections
NMC = IPC // P            # 4 inner chunks per core
NDO = D // P              # 8 output-dim chunks

F32 = mybir.dt.float32
F32R = mybir.dt.float32r
BF16 = mybir.dt.bfloat16
EXP = mybir.ActivationFunctionType.Exp


def _r(ap):
    return ap if ap.dtype == F32R else ap.bitcast(F32R)


def _build(loop=1):
    nc = bacc.Bacc("TRN2", target_bir_lowering=False, debug=False)
    xT = nc.dram_tensor("xT", [D, N], BF16, kind="ExternalInput")
    wq = nc.dram_tensor("wq", [D, IPC], BF16, kind="ExternalInput")
    wk = nc.dram_tensor("wk", [D, IPC], BF16, kind="ExternalInput")
    wv = nc.dram_tensor("wv", [D, IPC], BF16, kind="ExternalInput")
    wo = nc.dram_tensor("wo", [IPC, D], BF16, kind="ExternalInput")
    binmT = nc.dram_tensor("binmT", [N, N], BF16, kind="ExternalInput")
    outT = nc.dram_tensor("outT", [D, N], BF16, kind="ExternalOutput")

    xT_r = xT.rearrange("(kc p) n -> kc p n", p=P)
    wq_r = wq.rearrange("(kc p) m -> kc p m", p=P)
    wk_r = wk.rearrange("(kc p) m -> kc p m", p=P)
    wv_r = wv.rearrange("(kc p) m -> kc p m", p=P)
    wo_r = wo.rearrange("(kc p) m -> kc p m", p=P)
    binmT_r = binmT.rearrange("(jc p) i -> p jc i", p=P)
    outT_r = outT.rearrange("(do p) n -> do p n", p=P)

    with tile.TileContext(nc) as tc, ExitStack() as ctx:
        xt_pool = ctx.enter_context(tc.tile_pool(name="xt", bufs=2))
        w_pool = ctx.enter_context(tc.tile_pool(name="w", bufs=5))
        qk_pool = ctx.enter_context(tc.tile_pool(name="qk", bufs=2))
        v_pool = ctx.enter_context(tc.tile_pool(name="v", bufs=2))
        m_pool = ctx.enter_context(tc.tile_pool(name="m", bufs=2))
        p_pool = ctx.enter_context(tc.tile_pool(name="p", bufs=6))
        ot_pool = ctx.enter_context(tc.tile_pool(name="ot", bufs=4))
        stage_pool = ctx.enter_context(tc.tile_pool(name="stage", bufs=4))
        small_pool = ctx.enter_context(tc.tile_pool(name="small", bufs=3))
        bc_pool = ctx.enter_context(tc.tile_pool(name="bc", bufs=2))
        # PSUM: st 2x(2 banks) + u 2x(1 bank) + t 2x(1 bank) = 8 banks
        psum_st = ctx.enter_context(tc.tile_pool(name="st", bufs=2, space="PSUM"))
        psum_u = ctx.enter_context(tc.tile_pool(name="u", bufs=2, space="PSUM"))
        psum_t = ctx.enter_context(tc.tile_pool(name="t", bufs=2, space="PSUM"))

        if loop > 1:
            loop_cm = tc.For_i(0, loop, 1)
        else:
            loop_cm = None
        with (loop_cm if loop_cm is not None else nullcontext()):
            # ---------------- DMA in ----------------
            xt_s = xt_pool.tile([P, NKC, N], BF16, tag="xt")
            wq_s = w_pool.tile([P, NKC, IPC], BF16, tag="w")
            wk_s = w_pool.tile([P, NKC, IPC], BF16, tag="w")
            wv_s = w_pool.tile([P, NKC, IPC], BF16, tag="w")
            for kc in range(NKC):
                nc.sync.dma_start(xt_s[:, kc, :], xT_r[kc])
                nc.sync.dma_start(wv_s[:, kc, :], wv_r[kc])
            for kc in range(NKC):
                nc.sync.dma_start(wq_s[:, kc, :], wq_r[kc])
            for kc in range(NKC):
                nc.sync.dma_start(wk_s[:, kc, :], wk_r[kc])
            binm_s = m_pool.tile([P, NJ, N], BF16, tag="binm")
            for jc in range(NJ):
                nc.sync.dma_start(binm_s[:, jc, :], binmT_r[:, jc, :])

            qt_s = qk_pool.tile([P, NMC, N], BF16, tag="qt")
            kt_s = qk_pool.tile([P, NMC, N], BF16, tag="kt")
            v_s = v_pool.tile([P, NJ, HPC, DH + 1], BF16, tag="v")
            ones_col = small_pool.tile([P, 1], BF16, tag="onescol")
            nc.vector.memset(ones_col[:], 1.0)
            # trigger the exp ACT-table load (~1.3us) during the DMA phase
            warm = small_pool.tile([1, 1], F32, tag="actwarm")
            nc.scalar.activation(warm, ones_col[:1, :1], EXP)
            nc.vector.tensor_copy(
                v_s[:, :, :, DH],
                ones_col[:, :, None].to_broadcast([P, NJ, HPC]),
            )

            # wo DMA issued after binm; first use is proj(0) much later.
            wo_s = w_pool.tile([P, NMC, D], BF16, tag="w")
            for kc in range(NMC):
                nc.sync.dma_start(wo_s[:, kc, :], wo_r[kc])

            # ---------------- compute emitters ----------------
            def emit_v_dbl(t):
                """V projection for key blocks 2t and 2t+1 in one st-ring
                double tile (the st ring is otherwise idle pre-attention)."""
                pv = psum_st.tile([P, 2, IPC], F32, tag="st", name=f"pv_{t}")
                for two in range(2):
                    for kc in range(NKC):
                        nc.tensor.matmul(
                            pv[:, two],
                            lhsT=xt_s[:, kc, (2 * t + two) * P:(2 * t + two + 1) * P],
                            rhs=wv_s[:, kc, :],
                            start=(kc == 0),
                            stop=(kc == NKC - 1),
                        )
                nc.scalar.copy(
                    v_s[:, 2 * t:2 * t + 2, :, :DH],
                    pv.rearrange("p two (h d) -> p two h d", h=HPC),
                )

            def emit_qk_grp(m, i, which, copy_eng="dve"):
                """One projection group: qt or kt for pair m, block i."""
                isl = slice(i * IC, (i + 1) * IC)
                w_s, dst = (wq_s, qt_s) if which == "q" else (wk_s, kt_s)
                pg = psum_t.tile([P, IC], F32, tag="t", name=f"p{which}_{m}_{i}")
                for kc in range(NKC):
                    nc.tensor.matmul(
                        pg,
                        lhsT=w_s[:, kc, m * P:(m + 1) * P],
                        rhs=xt_s[:, kc, isl],
                        start=(kc == 0),
                        stop=(kc == NKC - 1),
                    )
                if copy_eng == "act":
                    nc.scalar.copy(dst[:, m, isl], pg)
                else:
                    nc.vector.tensor_copy(dst[:, m, isl], pg)

            ots = []
            for i in range(NI):
                ots.append(
                    ot_pool.tile([P, NMC, IC], BF16, tag="ot", name=f"ot_{i}")
                )

            def emit_attention(i, pair, at_jc=None):
                isl = slice(i * IC, (i + 1) * IC)
                ot = ots[i]
                us = [
                    psum_u.tile([P, IC], F32, tag="u", name=f"u_{i}_{pair}_{h}")
                    for h in range(2)
                ]
                for jc in range(NJ):
                    if at_jc is not None and jc in at_jc:
                        at_jc[jc]()
                    jsl = slice(jc * P, (jc + 1) * P)
                    st = psum_st.tile([P, 2, IC], F32, tag="st")
                    for half in range(2):
                        hsl = slice(half * DH, (half + 1) * DH)
                        nc.tensor.matmul(
                            st[:, half],
                            lhsT=kt_s[hsl, pair, jsl],
                            rhs=qt_s[hsl, pair, isl],
                            start=True,
                            stop=True,
                        )
                    p_t = p_pool.tile([P, 2, IC], BF16, tag="p")
                    nc.scalar.activation(p_t, st, EXP)
                    nc.vector.tensor_mul(
                        out=p_t, in0=p_t,
                        in1=binm_s[:, jc, None, isl].to_broadcast([P, 2, IC]),
                    )
                    for half in range(2):
                        h = 2 * pair + half
                        nc.tensor.matmul(
                            us[half][: DH + 1],
                            lhsT=v_s[:, jc, h, :],
                            rhs=p_t[:, half],
                            start=(jc == 0),
                            stop=(jc == NJ - 1),
                        )
                for half in range(2):
                    u = us[half]
                    rsum = small_pool.tile([1, IC], F32, tag="rsum")
                    nc.vector.tensor_copy(rsum, u[DH:DH + 1, :])
                    rr = small_pool.tile([1, IC], F32, tag="rr")
                    nc.vector.reciprocal_approx_fast(out=rr, in_=rsum)
                    bcs = bc_pool.tile([DH, IC], F32, tag="bcs")
                    nc.gpsimd.partition_broadcast(bcs, rr)
                    nc.vector.tensor_mul(
                        out=ot[half * DH:(half + 1) * DH, pair, :],
                        in0=u[:DH, :],
                        in1=bcs,
                    )

            def emit_proj_grp(i, do, stage_eng="act", ring="t"):
                isl = slice(i * IC, (i + 1) * IC)
                pool = psum_t if ring == "t" else psum_u
                pr = pool.tile([P, IC], F32, tag=("t" if ring == "t" else "u"),
                               name=f"pr_{i}_{do}")
                for kc in range(NMC):
                    nc.tensor.matmul(
                        pr,
                        lhsT=wo_s[:, kc, do * P:(do + 1) * P],
                        rhs=ots[i][:, kc, :],
                        start=(kc == 0),
                        stop=(kc == NMC - 1),
                    )
                stg = stage_pool.tile([P, IC], BF16, tag="stg")
                if stage_eng == "act":
                    nc.scalar.copy(stg, pr)
                else:
                    nc.vector.tensor_copy(stg, pr)
                nc.sync.dma_start(outT_r[do][:, isl], stg)

            # ---------------- schedule ----------------
            # Early phase: V doubles (st ring) interleaved with pair-0 QKV
            # groups (t ring); the emission order tracks the DMA stream.
            emit_v_dbl(0)
            emit_v_dbl(1)
            emit_v_dbl(2)
            emit_qk_grp(0, 0, "q", copy_eng="act")
            emit_v_dbl(3)
            emit_qk_grp(0, 1, "q", copy_eng="act")
            emit_qk_grp(0, 0, "k", copy_eng="act")
            emit_qk_grp(0, 1, "k", copy_eng="act")

            for pair in range(NMC):
                for i in range(NI):
                    hooks = {}
                    if pair + 1 < NMC:
                        hooks[4] = lambda m=pair + 1, ii=i: emit_qk_grp(m, ii, "q")
                        hooks[7] = lambda m=pair + 1, ii=i: emit_qk_grp(m, ii, "k")
                    elif i == 1:
                        # last block: proj(0) groups as late fillers
                        for jc, do in zip((2, 3, 4, 5, 6, 7), range(6)):
                            hooks[jc] = lambda d=do: emit_proj_grp(0, d, "dve", "t")
                    emit_attention(i, pair, at_jc=hooks)
            for do in range(6, NDO):
                emit_proj_grp(0, do, "act", "t")
            for do in range(NDO):
                emit_proj_grp(1, do, "act" if do % 2 == 0 else "dve",
                              "t" if do % 2 == 0 else "u")

    nc.compile()
    return nc


_nc_cache = {}


def _get_nc(loop=1):
    if loop not in _nc_cache:
        _nc_cache[loop] = _build(loop)
    return _nc_cache[loop]


_last_results = [None]
_last_in_maps = [None]


def kernel(x, mask, Wq, Wk, Wv, Wo, bo):
    x = np.asarray(x, dtype=np.float32)
    mask = np.asarray(mask)
    Wq = np.asarray(Wq, dtype=np.float32)
    Wk = np.asarray(Wk, dtype=np.float32)
    Wv = np.asarray(Wv, dtype=np.float32)
    Wo = np.asarray(Wo, dtype=np.float32)
    bo = np.asarray(bo, dtype=np.float32)

    nc = _get_nc()
    in_maps = []
    for c in range(NCORES):
        b, g = divmod(c, 2)
        gsl = slice(g * IPC, (g + 1) * IPC)
        keep = (mask[b, 0] == 0).T
        in_maps.append(
            {
                "xT": np.ascontiguousarray(x[b].T.astype(ml_dtypes.bfloat16)),
                "wq": np.ascontiguousarray((Wq[:, gsl] * np.float32(SCALE)).astype(ml_dtypes.bfloat16)),
                "wk": np.ascontiguousarray(Wk[:, gsl].astype(ml_dtypes.bfloat16)),
                "wv": np.ascontiguousarray(Wv[:, gsl].astype(ml_dtypes.bfloat16)),
                "wo": np.ascontiguousarray(Wo[gsl, :].astype(ml_dtypes.bfloat16)),
                "binmT": np.ascontiguousarray(keep.astype(ml_dtypes.bfloat16)),
            }
        )
    _last_in_maps[0] = in_maps
    res = run_bass_kernel_spmd(nc, in_maps, core_ids=list(range(NCORES)))
    _last_results[0] = res
    outs = [np.asarray(r["outT"], dtype=np.float32) for r in res.results]
    out = np.empty((B, N, D), np.float32)
    for b in range(B):
        out[b] = (outs[2 * b] + outs[2 * b + 1]).T + bo
    return out


# revision 19
# speedup vs baseline: 1.1061x; 1.0025x over previous
"""Multi-head attention (B=4, N=1024, D=1024, 16 heads x 64) on 8 TRN2 cores.

Sharding: core c -> (batch b = c//2, head-group g = c%2). Each core computes
attention for 8 heads of one batch plus its slice of the output projection
(Wo row-parallel); host sums the two head-group partials per batch.

Schedule (v3):
- DMA order: (xt,wv)x8, wq x8, wk x8, binm x8, wo — V projection only needs
  the stream head; attention starts right after wk lands.
- PSUM rings: st pool (2 x 2 banks) serves the early V-projection double
  tiles then the attention score tiles; u pool (2 x 1 bank) holds only the
  softmax accumulators; t pool (2 x 1 bank) takes every transient group
  (QKV projections, prefetch, output projection).
- Each engine executes its stream in order, so all filler work (QK prefetch
  for the next pair, proj(0) groups) is hooked late in each attention block
  (jc3/jc6) where the exp pipeline otherwise gates the PE.
- proj(0) overlaps attention(1,3); proj(1) drains on alternating psum rings
  with stages alternating Scalar/Vector engines. Output is bf16.
"""

from contextlib import ExitStack, nullcontext

import ml_dtypes
import numpy as np

import concourse.bass as bass
import concourse.mybir as mybir
import concourse.tile as tile
from concourse import bacc
from concourse.bass_utils import run_bass_kernel_spmd

B, N, D = 4, 1024, 1024
HEADS, DH = 16, 64
SCALE = DH ** -0.5
NCORES = 8
HPC = HEADS // 2          # heads per core = 8
IPC = HPC * DH            # inner slice per core = 512
P = 128
IC = 512                  # i (query position) chunk = max psum free dim
NI = N // IC              # 2
NJ = N // P               # 8 key-position chunks
NKC = D // P              # 8 contraction chunks for projections
NMC = IPC // P            # 4 inner chunks per core
NDO = D // P              # 8 output-dim chunks

F32 = mybir.dt.float32
F32R = mybir.dt.float32r
BF16 = mybir.dt.bfloat16
EXP = mybir.ActivationFunctionType.Exp


def _r(ap):
    return ap if ap.dtype == F32R else ap.bitcast(F32R)


def _build(loop=1):
    nc = bacc.Bacc("TRN2", target_bir_lowering=False, debug=False)
    xT = nc.dram_tensor("xT", [D, N], BF16, kind="ExternalInput")
    wq = nc.dram_tensor("wq", [D, IPC], BF16, kind="ExternalInput")
    wk = nc.dram_tensor("wk", [D, IPC], BF16, kind="ExternalInput")
    wv = nc.dram_tensor("wv", [D, IPC], BF16, kind="ExternalInput")
    wo = nc.dram_tensor("wo", [IPC, D], BF16, kind="ExternalInput")
    binmT = nc.dram_tensor("binmT", [N, N], BF16, kind="ExternalInput")
    outT = nc.dram_tensor("outT", [D, N], BF16, kind="ExternalOutput")

    xT_r = xT.rearrange("(kc p) n -> kc p n", p=P)
    wq_r = wq.rearrange("(kc p) m -> kc p m", p=P)
    wk_r = wk.rearrange("(kc p) m -> kc p m", p=P)
    wv_r = wv.rearrange("(kc p) m -> kc p m", p=P)
    wo_r = wo.rearrange("(kc p) m -> kc p m", p=P)
    binmT_r = binmT.rearrange("(jc p) i -> p jc i", p=P)
    outT_r = outT.rearrange("(do p) n -> do p n", p=P)

    with tile.TileContext(nc) as tc, ExitStack() as ctx:
        xt_pool = ctx.enter_context(tc.tile_pool(name="xt", bufs=2))
        w_pool = ctx.enter_context(tc.tile_pool(name="w", bufs=5))
        qk_pool = ctx.enter_context(tc.tile_pool(name="qk", bufs=2))
        v_pool = ctx.enter_context(tc.tile_pool(name="v", bufs=2))
        m_pool = ctx.enter_context(tc.tile_pool(name="m", bufs=2))
        p_pool = ctx.enter_context(tc.tile_pool(name="p", bufs=6))
        ot_pool = ctx.enter_context(tc.tile_pool(name="ot", bufs=4))
        stage_pool = ctx.enter_context(tc.tile_pool(name="stage", bufs=4))
        small_pool = ctx.enter_context(tc.tile_pool(name="small", bufs=3))
        bc_pool = ctx.enter_context(tc.tile_pool(name="bc", bufs=2))
        # PSUM: st 2x(2 banks) + u 2x(1 bank) + t 2x(1 bank) = 8 banks
        psum_st = ctx.enter_context(tc.tile_pool(name="st", bufs=2, space="PSUM"))
        psum_u = ctx.enter_context(tc.tile_pool(name="u", bufs=2, space="PSUM"))
        psum_t = ctx.enter_context(tc.tile_pool(name="t", bufs=2, space="PSUM"))

        if loop > 1:
            loop_cm = tc.For_i(0, loop, 1)
        else:
            loop_cm = None
        with (loop_cm if loop_cm is not None else nullcontext()):
            # ---------------- DMA in ----------------
            xt_s = xt_pool.tile([P, NKC, N], BF16, tag="xt")
            wq_s = w_pool.tile([P, NKC, IPC], BF16, tag="w")
            wk_s = w_pool.tile([P, NKC, IPC], BF16, tag="w")
            wv_s = w_pool.tile([P, NKC, IPC], BF16, tag="w")
            for kc in range(NKC):
                nc.sync.dma_start(xt_s[:, kc, :], xT_r[kc])
                nc.sync.dma_start(wv_s[:, kc, :], wv_r[kc])
            for kc in range(NKC):
                nc.sync.dma_start(wq_s[:, kc, :], wq_r[kc])
            for kc in range(NKC):
                nc.sync.dma_start(wk_s[:, kc, :], wk_r[kc])
            binm_s = m_pool.tile([P, NJ, N], BF16, tag="binm")
            for jc in range(NJ):
                nc.sync.dma_start(binm_s[:, jc, :], binmT_r[:, jc, :])

            qt_s = qk_pool.tile([P, NMC, N], BF16, tag="qt")
            kt_s = qk_pool.tile([P, NMC, N], BF16, tag="kt")
            v_s = v_pool.tile([P, NJ, HPC, DH + 1], BF16, tag="v")
            ones_col = small_pool.tile([P, 1], BF16, tag="onescol")
            nc.vector.memset(ones_col[:], 1.0)
            # trigger the exp ACT-table load (~1.3us) during the DMA phase
            warm = small_pool.tile([1, 1], F32, tag="actwarm")
            nc.scalar.activation(warm, ones_col[:1, :1], EXP)
            nc.vector.tensor_copy(
                v_s[:, :, :, DH],
                ones_col[:, :, None].to_broadcast([P, NJ, HPC]),
            )

            # wo DMA issued after binm; first use is proj(0) much later.
            wo_s = w_pool.tile([P, NMC, D], BF16, tag="w")
            for kc in range(NMC):
                nc.sync.dma_start(wo_s[:, kc, :], wo_r[kc])

            # ---------------- compute emitters ----------------
            def emit_v_dbl(t):
                """V projection for key blocks 2t and 2t+1 in one st-ring
                double tile (the st ring is otherwise idle pre-attention)."""
                pv = psum_st.tile([P, 2, IPC], F32, tag="st", name=f"pv_{t}")
                for two in range(2):
                    for kc in range(NKC):
                        nc.tensor.matmul(
                            pv[:, two],
                            lhsT=xt_s[:, kc, (2 * t + two) * P:(2 * t + two + 1) * P],
                            rhs=wv_s[:, kc, :],
                            start=(kc == 0),
                            stop=(kc == NKC - 1),
                        )
                nc.scalar.copy(
                    v_s[:, 2 * t:2 * t + 2, :, :DH],
                    pv.rearrange("p two (h d) -> p two h d", h=HPC),
                )

            def emit_qk_grp(m, i, which, copy_eng="dve"):
                """One projection group: qt or kt for pair m, block i."""
                isl = slice(i * IC, (i + 1) * IC)
                w_s, dst = (wq_s, qt_s) if which == "q" else (wk_s, kt_s)
                pg = psum_t.tile([P, IC], F32, tag="t", name=f"p{which}_{m}_{i}")
                for kc in range(NKC):
                    nc.tensor.matmul(
                        pg,
                        lhsT=w_s[:, kc, m * P:(m + 1) * P],
                        rhs=xt_s[:, kc, isl],
                        start=(kc == 0),
                        stop=(kc == NKC - 1),
                    )
                if copy_eng == "act":
                    nc.scalar.copy(dst[:, m, isl], pg)
                else:
                    nc.vector.tensor_copy(dst[:, m, isl], pg)

            ots = []
            for i in range(NI):
                ots.append(
                    ot_pool.tile([P, NMC, IC], BF16, tag="ot", name=f"ot_{i}")
                )

            def emit_attention(i, pair, at_jc=None):
                isl = slice(i * IC, (i + 1) * IC)
                ot = ots[i]
                us = [
                    psum_u.tile([P, IC], F32, tag="u", name=f"u_{i}_{pair}_{h}")
                    for h in range(2)
                ]
                for jc in range(NJ):
                    if at_jc is not None and jc in at_jc:
                        at_jc[jc]()
                    jsl = slice(jc * P, (jc + 1) * P)
                    st = psum_st.tile([P, 2, IC], F32, tag="st")
                    for half in range(2):
                        hsl = slice(half * DH, (half + 1) * DH)
                        nc.tensor.matmul(
                            st[:, half],
                            lhsT=kt_s[hsl, pair, jsl],
                            rhs=qt_s[hsl, pair, isl],
                            start=True,
                            stop=True,
                        )
                    p_t = p_pool.tile([P, 2, IC], BF16, tag="p")
                    nc.scalar.activation(p_t, st, EXP)
                    nc.vector.tensor_mul(
                        out=p_t, in0=p_t,
                        in1=binm_s[:, jc, None, isl].to_broadcast([P, 2, IC]),
                    )
                    for half in range(2):
                        h = 2 * pair + half
                        nc.tensor.matmul(
                            us[half][: DH + 1],
                            lhsT=v_s[:, jc, h, :],
                            rhs=p_t[:, half],
                            start=(jc == 0),
                            stop=(jc == NJ - 1),
                        )
                for half in range(2):
                    u = us[half]
                    rsum = small_pool.tile([1, IC], F32, tag="rsum")
                    nc.vector.tensor_copy(rsum, u[DH:DH + 1, :])
                    rr = small_pool.tile([1, IC], F32, tag="rr")
                    nc.vector.reciprocal_approx_fast(out=rr, in_=rsum)
                    bcs = bc_pool.tile([DH, IC], F32, tag="bcs")
                    nc.gpsimd.partition_broadcast(bcs, rr)
                    nc.vector.tensor_mul(
                        out=ot[half * DH:(half + 1) * DH, pair, :],
                        in0=u[:DH, :],
                        in1=bcs,
                    )

            def emit_proj_grp(i, do, stage_eng="act", ring="t"):
                isl = slice(i * IC, (i + 1) * IC)
                pool = psum_t if ring == "t" else psum_u
                pr = pool.tile([P, IC], F32, tag=("t" if ring == "t" else "u"),
                               name=f"pr_{i}_{do}")
                for kc in range(NMC):
                    nc.tensor.matmul(
                        pr,
                        lhsT=wo_s[:, kc, do * P:(do + 1) * P],
                        rhs=ots[i][:, kc, :],
                        start=(kc == 0),
                        stop=(kc == NMC - 1),
                    )
                stg = stage_pool.tile([P, IC], BF16, tag="stg")
                if stage_eng == "act":
                    nc.scalar.copy(stg, pr)
                else:
                    nc.vector.tensor_copy(stg, pr)
                nc.sync.dma_start(outT_r[do][:, isl], stg)

            # ---------------- schedule ----------------
            # Early phase: V doubles (st ring) interleaved with pair-0 QKV
            # groups (t ring); the emission order tracks the DMA stream.
            emit_v_dbl(0)
            emit_v_dbl(1)
            emit_v_dbl(2)
            emit_qk_grp(0, 0, "q", copy_eng="act")
            emit_v_dbl(3)
            emit_qk_grp(0, 1, "q", copy_eng="act")
            emit_qk_grp(0, 0, "k", copy_eng="act")
            emit_qk_grp(0, 1, "k", copy_eng="act")

            for pair in range(NMC):
                for i in range(NI):
                    hooks = {}
                    if pair + 1 < NMC:
                        hooks[4] = lambda m=pair + 1, ii=i: emit_qk_grp(m, ii, "q")
                        if not (pair == NMC - 2 and i == 1):
                            hooks[7] = lambda m=pair + 1, ii=i: emit_qk_grp(m, ii, "k")
                    elif i == 0:
                        # block (0,3): filler = the last prefetch group,
                        # moved here from block (1,2); early enough that this
                        # block's own jc>=4 score matmuls see the kt columns
                        hooks[1] = lambda: emit_qk_grp(NMC - 1, 1, "k")
                    elif i == 1:
                        # last block: proj(0) groups as late fillers
                        for jc, do in zip((2, 3, 4, 5, 6, 7), range(6)):
                            hooks[jc] = lambda d=do: emit_proj_grp(0, d, "dve", "t")
                    emit_attention(i, pair, at_jc=hooks)
            for do in range(6, NDO):
                emit_proj_grp(0, do, "act", "t")
            for do in range(NDO):
                emit_proj_grp(1, do, "act" if do % 2 == 0 else "dve",
                              "t" if do % 2 == 0 else "u")

    nc.compile()
    return nc


_nc_cache = {}


def _get_nc(loop=1):
    if loop not in _nc_cache:
        _nc_cache[loop] = _build(loop)
    return _nc_cache[loop]


_last_results = [None]
_last_in_maps = [None]


def kernel(x, mask, Wq, Wk, Wv, Wo, bo):
    x = np.asarray(x, dtype=np.float32)
    mask = np.asarray(mask)
    Wq = np.asarray(Wq, dtype=np.float32)
    Wk = np.asarray(Wk, dtype=np.float32)
    Wv = np.asarray(Wv, dtype=np.float32)
    Wo = np.asarray(Wo, dtype=np.float32)
    bo = np.asarray(bo, dtype=np.float32)

    nc = _get_nc()
    in_maps = []
    for c in range(NCORES):
        b, g = divmod(c, 2)
        gsl = slice(g * IPC, (g + 1) * IPC)
        keep = (mask[b, 0] == 0).T
        in_maps.append(
            {
                "xT": np.ascontiguousarray(x[b].T.astype(ml_dtypes.bfloat16)),
                "wq": np.ascontiguousarray((Wq[:, gsl] * np.float32(SCALE)).astype(ml_dtypes.bfloat16)),
                "wk": np.ascontiguousarray(Wk[:, gsl].astype(ml_dtypes.bfloat16)),
                "wv": np.ascontiguousarray(Wv[:, gsl].astype(ml_dtypes.bfloat16)),
                "wo": np.ascontiguousarray(Wo[gsl, :].astype(ml_dtypes.bfloat16)),
                "binmT": np.ascontiguousarray(keep.astype(ml_dtypes.bfloat16)),
            }
        )
    _last_in_maps[0] = in_maps
    res = run_bass_kernel_spmd(nc, in_maps, core_ids=list(range(NCORES)))
    _last_results[0] = res
    outs = [np.asarray(r["outT"], dtype=np.float32) for r in res.results]
    out = np.empty((B, N, D), np.float32)
    for b in range(B):
        out[b] = (outs[2 * b] + outs[2 * b + 1]).T + bo
    return out
